# revision 27
# baseline (speedup 1.0000x reference)
"""Trainium2 Bass kernel for the 2-layer heterogeneous GCN encoder.

v6 strategy (8 NeuronCores, SPMD, dst-sharded):
  - Core k owns user rows [k*12500,(k+1)*12500) and item rows
    [k*6250,(k+1)*6250); edges are routed to their dst owner.
  - Full fp16 node tables are replicated to every core as inputs (no
    input AllGathers); layer-1 outputs are AllGathered (2 collectives).
  - (window, chunk)-major edge order with the window aggregate resident
    in PSUM across its chunks: no SBUF accumulator, no re-inject.
  - Aggregate-then-transform: segment_sum(x[src]*norm, dst) @ W with the
    per-window segment sum done as PE matmuls against an on-chip one-hot
    S[e, col] = (dstcol[e] == col) * norm[e]. S tiles are built on BOTH
    the DVE (2-op tensor_scalar, is_equal*mult) and the Scalar engine
    (Abs -> Relu(1-t) -> Copy*norm-ptr), split K_ACTS:1, so neither
    engine is the wall; per-instruction cost dominates (flat in width).
  - One-window software lookahead: window w's psum->SBUF copies + flush
    (W matmuls -> Act bias/relu -> PE transpose -> Act copy -> DMA) are
    emitted after window w+1's chains, so queue-head blocking never
    starves the S-build streams.
  - src rows fetched with <=1024-row SWDGE dma_gather calls per
    (window, chunk) cell (int16 idx; >1024 idx/call wedges the ucode).

Self-contained: hardcodes shapes; host does index prep (degrees/norms,
sharding, (window,chunk) sort, int16 packing, fp16 casts).
"""

import os
import sys

sys.path.insert(0, "/opt/trn_rl_repo")

import numpy as np

import concourse.bass as bass
import concourse.bacc as bacc
import concourse.mybir as mybir
import concourse.tile as tile
from concourse.bass_utils import run_bass_kernel_spmd

P = 128
NCORES = 8
F32 = mybir.dt.float32
F16 = mybir.dt.float16
I16 = mybir.dt.int16

CFG = dict(N_U=100000, N_I=50000, E=1600000, D=128)
WIN = int(os.environ.get("K_WIN", "384"))
CHUNK = 32768
CALLT = 8      # tiles per dma_gather call (1024-idx ucode cap)
NQ = int(os.environ.get("K_NQ", "4"))
K_ACTS = int(os.environ.get("K_ACTS", "2"))  # every K-th S-build on Act (0=off)
K_PP = int(os.environ.get("K_PP", "4"))


def _cdiv(a, b):
    return (a + b - 1) // b


def prep_relation(src, dst, n_src, n_dst, ncores=NCORES, win=WIN, chunk=CHUNK,
                  callt=CALLT):
    """Shard edges by dst owner, sort by (dst-window, src-chunk), pad each
    (window,chunk) run to whole 128-edge tiles harmonized across cores.

    Returns (sched dict, per-core [idx16, dstw32, norm32])."""
    shard = n_dst // ncores
    nwin = _cdiv(shard, win)
    nchunk = _cdiv(n_src, chunk)

    deg_s = np.bincount(src, minlength=n_src).astype(np.float64)
    deg_d = np.bincount(dst, minlength=n_dst).astype(np.float64)
    inv_s = np.where(deg_s > 0, 1.0 / np.sqrt(deg_s), 0.0)
    inv_d = np.where(deg_d > 0, 1.0 / np.sqrt(deg_d), 0.0)
    norm = (inv_s[src] * inv_d[dst]).astype(np.float32)

    owner = dst // shard
    counts = np.zeros((ncores, nwin, nchunk), np.int64)
    per_core = []
    for k in range(ncores):
        sel = owner == k
        s_k = src[sel]
        d_k = dst[sel] - k * shard
        n_k = norm[sel]
        key = (d_k // win) * nchunk + (s_k // chunk)
        order = np.argsort(key, kind="stable")
        s_k, d_k, n_k = s_k[order], d_k[order], n_k[order]
        counts[k] = np.bincount(key[order], minlength=nwin * nchunk).reshape(
            nwin, nchunk
        )
        per_core.append((s_k, d_k, n_k))

    Twc = -(-counts.max(axis=0) // P)          # [nwin, nchunk] tiles
    base_wc = np.zeros((nwin, nchunk), np.int64)
    flat = Twc.reshape(-1)
    base_wc.reshape(-1)[:] = np.concatenate([[0], np.cumsum(flat)[:-1]])
    Ttot = max(int(Twc.sum()), 1)

    sched = dict(
        nwin=nwin, nchunk=nchunk, shard=shard,
        Twc=Twc.tolist(), base_wc=base_wc.tolist(), Ttot=Ttot,
    )

    packed = []
    for k in range(ncores):
        s_k, d_k, n_k = per_core[k]
        idxw = np.zeros((16, Ttot * 8), np.int16)
        dstw = np.full((P, Ttot), -1.0, np.float32)
        nrm = np.zeros((P, Ttot), np.float32)
        cnt = counts[k]
        starts = np.concatenate([[0], np.cumsum(cnt.ravel())[:-1]])
        tok = np.arange(len(s_k)) - np.repeat(starts, cnt.ravel())
        c_e = s_k // chunk
        w_e = d_k // win
        t_cell = tok // P                          # tile index within cell
        t_stream = base_wc[w_e, c_e] + t_cell      # global stream tile
        p = tok % P
        dstw[p, t_stream] = (d_k % win).astype(np.float32)
        nrm[p, t_stream] = n_k
        call_base = base_wc[w_e, c_e] + (t_cell // callt) * callt
        j = (t_cell % callt) * P + p               # position within call
        col = call_base * 8 + j // 16
        idxw[j % 16, col] = (s_k - c_e * chunk).astype(np.int16)
        packed.append((np.tile(idxw, (8, 1)), dstw, nrm, -nrm))
    return sched, packed


def build_program(cfg, scheds, win=WIN, chunk=CHUNK, callt=CALLT):
    N_U, N_I, D = cfg["N_U"], cfg["N_I"], cfg["D"]
    SU, SI = N_U // NCORES, N_I // NCORES
    NWU, NWI = _cdiv(SU, win), _cdiv(SI, win)

    ABL_NOS = os.environ.get("ABL_NOS") == "1"
    ABL_NOGATHER = os.environ.get("ABL_NOGATHER") == "1"
    ABL_NOFLUSH = os.environ.get("ABL_NOFLUSH") == "1"

    nc = bacc.Bacc("TRN2", target_bir_lowering=False, num_swdge_queues=NQ)

    xu_in = nc.dram_tensor("xu16", [N_U, D], F16, kind="ExternalInput")
    xi_in = nc.dram_tensor("xi16", [N_I, D], F16, kind="ExternalInput")
    W16in = {
        n: nc.dram_tensor(f"{n}_h", [D, D], F16, kind="ExternalInput")
        for n in ["W1_follows", "W1_rates", "W1_rev",
                  "W2_follows", "W2_rates", "W2_rev"]
    }
    bs = {
        n: nc.dram_tensor(n, [D], F32, kind="ExternalInput")
        for n in ["b1_follows", "b1_rates", "b1_rev",
                  "b2_follows", "b2_rates", "b2_rev"]
    }
    iota_in = nc.dram_tensor("iota16", [P, win], F16, kind="ExternalInput")
    ident_in = nc.dram_tensor("ident", [P, P], F32, kind="ExternalInput")
    streams = {}
    for r, sc in scheds.items():
        streams[r] = dict(
            idx=nc.dram_tensor(f"idx_{r}", [P, sc["Ttot"] * 8], I16,
                               kind="ExternalInput"),
            dstw=nc.dram_tensor(f"dstw_{r}", [P, sc["Ttot"]], F32,
                                kind="ExternalInput"),
            norm=nc.dram_tensor(f"norm_{r}", [P, sc["Ttot"]], F32,
                                kind="ExternalInput"),
            nneg=nc.dram_tensor(f"nneg_{r}", [P, sc["Ttot"]], F32,
                                kind="ExternalInput"),
        )
    out_user = nc.dram_tensor("out_user", [SU, D], F16, kind="ExternalOutput")
    out_item = nc.dram_tensor("out_item", [SI, D], F16, kind="ExternalOutput")

    qctr = [0]

    def next_q():
        q = qctr[0] % NQ
        qctr[0] += 1
        return q

    sctr = [0]  # global S-build counter for the DVE/Act split

    with tile.TileContext(nc) as tc:
        with (
            tc.tile_pool(name="const", bufs=1) as cp,
            tc.tile_pool(name="gp", bufs=10) as gp,
            tc.tile_pool(name="Sp", bufs=24) as sp,
            tc.tile_pool(name="atp", bufs=12) as atp,
            tc.tile_pool(name="aggp", bufs=6) as aggp,
            tc.tile_pool(name="hp", bufs=4) as hp,
            tc.tile_pool(name="outp", bufs=6) as outp,
            tc.tile_pool(name="ps", bufs=K_PP, space="PSUM") as pp,
            tc.tile_pool(name="ps2", bufs=2, space="PSUM") as pp2,
            tc.tile_pool(name="pstr", bufs=2, space="PSUM") as ptr,
            tc.tile_pool(name="dram", bufs=1, space="DRAM") as dp,
        ):
            # ---- constants ----
            iota_t = cp.tile([P, win], F16, tag="iota")
            nc.sync.dma_start(iota_t[:], iota_in[:])
            ident_t = cp.tile([P, P], F32, tag="ident")
            nc.sync.dma_start(ident_t[:], ident_in[:])
            W_t = {}
            for n, W in W16in.items():
                W_t[n] = cp.tile([P, P], F16, tag=f"W_{n}", name=f"W_{n}")
                nc.sync.dma_start(W_t[n][:], W[:])
            b_t = {}
            for n, b in bs.items():
                b_t[n] = cp.tile([P, 1], F32, tag=f"b_{n}", name=f"bt_{n}")
                nc.sync.dma_start(b_t[n][:], b[:].unsqueeze(1))
            buv = {}
            for l in (1, 2):
                buv[l] = cp.tile([P, 1], F32, tag=f"b{l}uv", name=f"b{l}uv")
                nc.vector.tensor_tensor(
                    out=buv[l][:], in0=b_t[f"b{l}_follows"][:],
                    in1=b_t[f"b{l}_rev"][:], op=mybir.AluOpType.add,
                )
                nc.vector.tensor_scalar_mul(buv[l][:], buv[l][:], 0.5)
            st = {}
            ixt = {}
            for r, sc in scheds.items():
                st[r] = {}
                for a in ("dstw", "norm", "nneg"):
                    st[r][a] = cp.tile([P, sc["Ttot"]], F32, tag=f"{a}_{r}",
                                       name=f"{a}t_{r}")
                    nc.sync.dma_start(st[r][a][:], streams[r][a][:])
                ixt[r] = cp.tile([P, sc["Ttot"] * 8], I16, tag=f"ix_{r}",
                                 name=f"ixt_{r}")
                nc.sync.dma_start(ixt[r][:], streams[r]["idx"][:])
            zc = cp.tile([P, win], F16, tag="zc")
            nc.vector.memset(zc[:], 0.0)

            # ---- DRAM layer-2 tables ----
            u_slice = dp.tile([SU, D], F16, tag="u_slice")
            it_slice = dp.tile([SI, D], F16, tag="it_slice")
            u_full = dp.tile([N_U, D], F16, tag="u_full", addr_space="Shared")
            it_full = dp.tile([N_I, D], F16, tag="it_full", addr_space="Shared")

            def build_S(rel, t_glob):
                if ABL_NOS:
                    return iota_t
                dptr = st[rel]["dstw"][:, t_glob : t_glob + 1]
                nptr = st[rel]["norm"][:, t_glob : t_glob + 1]
                sctr[0] += 1
                # K_ACTS>0: every K-th build on Act; K_ACTS<0: every |K|-th
                # build on DVE, the rest on Act.
                on_act = (K_ACTS > 0 and sctr[0] % K_ACTS == 0) or (
                    K_ACTS < 0 and sctr[0] % (-K_ACTS) != 0)
                if on_act:
                    nnptr = st[rel]["nneg"][:, t_glob : t_glob + 1]
                    t1 = atp.tile([P, win], F16, tag="at1")
                    nc.scalar.activation(
                        out=t1[:], in_=iota_t[:],
                        func=mybir.ActivationFunctionType.Abs,
                        bias=dptr, scale=-1.0,
                    )
                    # Relu(-norm*t1 + norm): norm at match (t1==0), else 0
                    Sg = sp.tile([P, win], F16, tag="S")
                    nc.scalar.activation(
                        out=Sg[:], in_=t1[:],
                        func=mybir.ActivationFunctionType.Relu,
                        bias=nptr, scale=nnptr,
                    )
                    return Sg
                Sg = sp.tile([P, win], F16, tag="S")
                nc.vector.tensor_scalar(
                    out=Sg[:], in0=iota_t[:],
                    scalar1=dptr, scalar2=nptr,
                    op0=mybir.AluOpType.is_equal,
                    op1=mybir.AluOpType.mult,
                )
                return Sg

            def window_calls(rel, w):
                """(c, t0, L) gather-call list for one relation's window."""
                sc = scheds[rel]
                calls = []
                for c in range(sc["nchunk"]):
                    nt = sc["Twc"][w][c]
                    base = sc["base_wc"][w][c]
                    for k0 in range(0, nt, callt):
                        calls.append((c, base + k0, min(callt, nt - k0)))
                return calls

            def emit_call(rel, table_ap, table_rows, ps, call, state):
                """One gather call + its S-builds/matmuls. state=[done, ntw]."""
                c, t0, L = call
                gbuf = gp.tile([P, callt, P], F16, tag="g")
                if not ABL_NOGATHER:
                    nc.gpsimd.dma_gather(
                        gbuf[:, :L, :],
                        table_ap[
                            c * chunk : min((c + 1) * chunk, table_rows), :
                        ],
                        ixt[rel][:, t0 * 8 : (t0 + L) * 8],
                        L * P,
                        L * P,
                        D,
                        elem_step=D,
                        queue_num=next_q(),
                    )
                for j in range(L):
                    Sg = build_S(rel, t0 + j)
                    nc.tensor.matmul(
                        out=ps[:],
                        lhsT=gbuf[:, j, :] if not ABL_NOGATHER
                        else iota_t[:, :P],
                        rhs=Sg[:],
                        start=(state[0] == 0),
                        stop=(state[0] == state[1] - 1),
                    )
                    state[0] += 1

            def emit_window_chain(rel, w, table_ap, table_rows, ps):
                """All (w, c) cells of one relation into one psum chain.
                Returns number of tiles emitted (0 => ps untouched)."""
                calls = window_calls(rel, w)
                ntw = sum(L for _, _, L in calls)
                if ntw == 0:
                    return 0
                state = [0, ntw]
                for call in calls:
                    emit_call(rel, table_ap, table_rows, ps, call, state)
                return ntw

            def emit_window_pair(w, tabU, rowsU, psF, tabI, rowsI, psV):
                """Interleave follows/rev gather calls of one user window so
                both relations' gathers issue early and PE stalls on one
                chain overlap the other chain's ready matmuls."""
                cF = window_calls("follows", w)
                cV = window_calls("rev", w)
                ntF = sum(L for _, _, L in cF)
                ntV = sum(L for _, _, L in cV)
                stF, stV = [0, ntF], [0, ntV]
                for i in range(max(len(cF), len(cV))):
                    if i < len(cF):
                        emit_call("follows", tabU, rowsU, psF, cF[i], stF)
                    if i < len(cV):
                        emit_call("rev", tabI, rowsI, psV, cV[i], stV)
                return ntF, ntV

            def agg_copy(ps, ntw):
                if ntw == 0:
                    return zc
                agg = aggp.tile([P, win], F16, tag="agg")
                nc.scalar.activation(
                    out=agg[:], in_=ps[:],
                    func=mybir.ActivationFunctionType.Copy,
                )
                return agg

            def write_block(h, dst_ap, w, nrows):
                for blk in range(_cdiv(nrows, P)):
                    r0, r1 = blk * P, min((blk + 1) * P, nrows)
                    pt = ptr.tile([P, P], F32, tag="ptr")
                    nc.tensor.transpose(
                        out=pt[: r1 - r0, :], in_=h[:, r0:r1],
                        identity=ident_t[:],
                    )
                    ob = outp.tile([P, P], F16, tag="ob")
                    nc.scalar.activation(
                        out=ob[: r1 - r0, :], in_=pt[: r1 - r0, :],
                        func=mybir.ActivationFunctionType.Copy,
                    )
                    nc.sync.dma_start(
                        dst_ap[w * win + r0 : w * win + r1, :], ob[: r1 - r0, :]
                    )

            def flush_user(l, w, aggF, aggV, dst_ap):
                if ABL_NOFLUSH:
                    return
                ph = pp2.tile([P, win], F32, tag="phps")
                nc.tensor.matmul(out=ph[:], lhsT=W_t[f"W{l}_follows"][:],
                                 rhs=aggF[:], start=True, stop=False)
                nc.tensor.matmul(out=ph[:], lhsT=W_t[f"W{l}_rev"][:],
                                 rhs=aggV[:], start=False, stop=True)
                h = hp.tile([P, win], F32, tag="h")
                nc.scalar.activation(
                    out=h[:], in_=ph[:],
                    func=mybir.ActivationFunctionType.Relu if l == 1
                    else mybir.ActivationFunctionType.Identity,
                    bias=buv[l][:], scale=0.5,
                )
                write_block(h, dst_ap, w, min(win, SU - w * win))

            def flush_item(l, w, aggR, dst_ap):
                if ABL_NOFLUSH:
                    return
                ph = pp2.tile([P, win], F32, tag="phps")
                nc.tensor.matmul(out=ph[:], lhsT=W_t[f"W{l}_rates"][:],
                                 rhs=aggR[:], start=True, stop=True)
                h = hp.tile([P, win], F32, tag="h")
                nc.scalar.activation(
                    out=h[:], in_=ph[:],
                    func=mybir.ActivationFunctionType.Relu if l == 1
                    else mybir.ActivationFunctionType.Identity,
                    bias=b_t[f"b{l}_rates"][:], scale=1.0,
                )
                write_block(h, dst_ap, w, min(win, SI - w * win))

            def user_pass(l, tabU, rowsU, tabI, rowsI, dst_ap):
                pending = None
                for w in range(NWU):
                    psF = pp.tile([P, win], F32, tag="runps")
                    psV = pp.tile([P, win], F32, tag="runps")
                    ntF, ntV = emit_window_pair(w, tabU, rowsU, psF,
                                                tabI, rowsI, psV)
                    if pending is not None:
                        flush_user(l, pending[0], pending[1], pending[2],
                                   dst_ap)
                    pending = (w, agg_copy(psF, ntF), agg_copy(psV, ntV))
                flush_user(l, pending[0], pending[1], pending[2], dst_ap)

            def item_pass(l, tabU, rowsU, dst_ap):
                pending = None
                for w in range(NWI):
                    psR = pp.tile([P, win], F32, tag="runps")
                    ntR = emit_window_chain("rates", w, tabU, rowsU, psR)
                    if pending is not None:
                        flush_item(l, pending[0], pending[1], dst_ap)
                    pending = (w, agg_copy(psR, ntR))
                flush_item(l, pending[0], pending[1], dst_ap)

            # ---- layer 1 ----
            user_pass(1, xu_in.ap(), N_U, xi_in.ap(), N_I, u_slice)
            nc.gpsimd.collective_compute(
                "AllGather", mybir.AluOpType.bypass,
                replica_groups=[list(range(NCORES))],
                ins=[u_slice[:]], outs=[u_full[:]],
            )
            item_pass(1, xu_in.ap(), N_U, it_slice)
            nc.gpsimd.collective_compute(
                "AllGather", mybir.AluOpType.bypass,
                replica_groups=[list(range(NCORES))],
                ins=[it_slice[:]], outs=[it_full[:]],
            )
            # ---- layer 2 (rates first: only needs u_full) ----
            item_pass(2, u_full, N_U, out_item.ap())
            user_pass(2, u_full, N_U, it_full, N_I, out_user.ap())

    nc.compile()
    return nc


def prepare(inputs, cfg=None, win=WIN, chunk=CHUNK, callt=CALLT):
    """Host-side prep + program build. Returns (nc, in_maps)."""
    if cfg is None:
        cfg = dict(CFG)
    N_U = inputs["x_user"].shape[0]
    N_I = inputs["x_item"].shape[0]
    cfg.update(N_U=N_U, N_I=N_I, E=len(inputs["follows_src"]))

    rel_edges = {
        "follows": (inputs["follows_src"], inputs["follows_dst"], N_U, N_U),
        "rates": (inputs["rates_src"], inputs["rates_dst"], N_U, N_I),
        "rev": (inputs["rev_src"], inputs["rev_dst"], N_I, N_U),
    }
    scheds, packs = {}, {}
    for r, (s, d, ns, nd) in rel_edges.items():
        sched, packed = prep_relation(
            np.asarray(s), np.asarray(d), ns, nd,
            win=win, chunk=chunk, callt=callt,
        )
        scheds[r] = sched
        packs[r] = packed

    nc = build_program(cfg, scheds, win=win, chunk=chunk, callt=callt)

    common = {
        "xu16": np.asarray(inputs["x_user"]).astype(np.float16),
        "xi16": np.asarray(inputs["x_item"]).astype(np.float16),
        "iota16": np.broadcast_to(
            np.arange(win, dtype=np.float16), (P, win)
        ).copy(),
        "ident": np.eye(P, dtype=np.float32),
    }
    for n in ["W1_follows", "W1_rates", "W1_rev",
              "W2_follows", "W2_rates", "W2_rev"]:
        common[f"{n}_h"] = np.asarray(inputs[n]).astype(np.float16)
    for n in ["b1_follows", "b1_rates", "b1_rev",
              "b2_follows", "b2_rates", "b2_rev"]:
        common[n] = np.asarray(inputs[n]).astype(np.float32)

    in_maps = []
    for k in range(NCORES):
        m = dict(common)
        for r in rel_edges:
            idxw, dstw, nrm, nneg = packs[r][k]
            m[f"idx_{r}"] = idxw
            m[f"dstw_{r}"] = dstw
            m[f"norm_{r}"] = nrm
            m[f"nneg_{r}"] = nneg
        in_maps.append(m)
    return nc, in_maps


def assemble(results):
    u2 = np.concatenate([results[k]["out_user"] for k in range(NCORES)], axis=0)
    i2 = np.concatenate([results[k]["out_item"] for k in range(NCORES)], axis=0)
    return np.concatenate([u2, i2], axis=0).astype(np.float32)


def kernel(**inputs):
    nc, in_maps = prepare(inputs)
    res = run_bass_kernel_spmd(nc, in_maps, list(range(NCORES)))
    return assemble(res.results)


if __name__ == "__main__":
    pass
